# revision 1
# baseline (speedup 1.0000x reference)
"""Trainium2 Bass kernel for OldNeighborhoodEncoder (segment_reduce).

Math (reference):
    fc1    = relu(X @ W1.T + b1)            # [N, 64], X = [N, 3]
    pooled = segment_max(fc1, cluster, S)   # [S, 64], cluster = arange(N)//32
    h      = relu(pooled @ W1g.T + b1g)     # [S, 64]
    out    = relu(h @ W2g.T + b2g)          # [S, 128]

Hardcoded sizes: N=1048576, S=32768 (32 pts/cluster), FEATURE=64, FG0=64,
FG1=128, 8 cores. Data-parallel over points: core d handles points
[d*131072, (d+1)*131072) == clusters [d*4096, (d+1)*4096); no collectives.

Device layout (per core):
  xt [6, 65536]: col c = 512*g + o (g in 0..127, o in 0..511); rows 0-2 =
    xyz of point 1024*g + o, rows 3-5 = xyz of point 1024*g + 512 + o.
  wpack [6,128] = blockdiag(W1.T, W1.T): one matmul column-block computes
    fc1 (pre-bias) for TWO 512-point chunks at once -> full 128-partition
    PE output. Bias+relu are deferred past the max (monotone).
  psum [128,4,16,32]: bank b holds g = 4i+b; view [.., q, t] with o=32q+t,
    so a single DVE reduce over t pools 4*16 = 64 cluster-halves.
  pooled [128, 32, 4, 16]: pooled[64a+f, i, b, q] = max_z of cluster
    128i + 32b + 16a + q, feature f.
  Tail: relu(+b1) -> blockdiag(W1g.T) matmul -> relu(+b1g) ->
    W2g.T matmul (K=64, separately for a=0 from partitions 0:64 and a=1
    from 64:128) -> relu(+b2g) -> outA/outB [128, 2048].

v1.5 perf structure: the main loop is DVE-reduce-bound (Pool/GPSIMD has no
legal max op on this target, so DVE does all 32 chunk reductions); weight
DMAs go on the Scalar queue (HWDGE; gpsimd SWDGE blocked the first matmul
~7us); relu(+b1) of pooled happens in slices during the main loop on ACT;
the tail MLP is pipelined in 512-col sub-slices with relu work split
between ACT and DVE, and output DMAs are split in halves on two queues.
"""

import sys
import numpy as np

if "/opt/trn_rl_repo" not in sys.path:
    sys.path.insert(0, "/opt/trn_rl_repo")

N = 1048576
S = 32768
PTS_PER_CLUSTER = 32
FEATURE = 64
FG0 = 64
FG1 = 128
NCORES = 8
NPC = N // NCORES          # 131072 points per core
SPC = S // NCORES          # 4096 clusters per core
G = NPC // 1024            # 128 column-groups of 512
NCHUNK = 32                # psum chunks per core (each = 4 groups)

USE_F32R = True

_PROGRAM = None  # (nc, input_names) cache


def _build_program():
    from concourse import bacc, bass, tile

    mybir = bass.mybir
    f32 = mybir.dt.float32
    # float32r: fp32 bits, full-rate (1 cycle/row) PE mode. The BIR verifier
    # requires every producer of an f32r matmul operand to emit f32r, so the
    # DRAM tensors / SBUF tiles on matmul paths are declared f32r outright.
    fmm = mybir.dt.float32r if USE_F32R else f32
    AX = mybir.AxisListType

    nc = bacc.Bacc("TRN2", target_bir_lowering=False, debug=False)

    xt = nc.dram_tensor("xt", [6, G * 512], fmm, kind="ExternalInput").ap()
    wpack = nc.dram_tensor("wpack", [6, 128], fmm, kind="ExternalInput").ap()
    b1d = nc.dram_tensor("b1d", [128, 1], f32, kind="ExternalInput").ap()
    w1gbd = nc.dram_tensor("w1gbd", [128, 128], fmm, kind="ExternalInput").ap()
    b1gd = nc.dram_tensor("b1gd", [128, 1], f32, kind="ExternalInput").ap()
    w2gt = nc.dram_tensor("w2gt", [128, 128], fmm, kind="ExternalInput").ap()
    b2g = nc.dram_tensor("b2g", [128, 1], f32, kind="ExternalInput").ap()
    outA = nc.dram_tensor("outA", [128, 2048], f32, kind="ExternalOutput").ap()
    outB = nc.dram_tensor("outB", [128, 2048], f32, kind="ExternalOutput").ap()

    with tile.TileContext(nc) as tc:
        with (
            tc.tile_pool(name="w", bufs=1) as wp,
            tc.tile_pool(name="x", bufs=3) as xp,
            tc.tile_pool(name="acc", bufs=1) as accp,
            tc.tile_pool(name="ps", bufs=2, space=bass.MemorySpace.PSUM) as pp,
        ):
            wpack_t = wp.tile([6, 128], fmm, tag="wpack")
            b1d_t = wp.tile([128, 1], f32, tag="b1d")
            w1gbd_t = wp.tile([128, 128], fmm, tag="w1gbd")
            b1gd_t = wp.tile([128, 1], f32, tag="b1gd")
            w2gt_t = wp.tile([128, 128], fmm, tag="w2gt")
            b2g_t = wp.tile([128, 1], f32, tag="b2g")
            # weight DMAs on the Scalar queue (HWDGE); wpack first — it
            # gates the first matmul.
            for t, d in (
                (wpack_t, wpack),
                (b1d_t, b1d),
                (w1gbd_t, w1gbd),
                (b1gd_t, b1gd),
                (w2gt_t, w2gt),
                (b2g_t, b2g),
            ):
                nc.scalar.dma_start(t[:], d[:])

            pooled = accp.tile([128, NCHUNK, 4, 16], f32, tag="pooled")
            pooledR = accp.tile([128, 2048], fmm, tag="pooledR")

            # main loop: fc1 matmuls + segment-max pooling
            for k in range(8):  # 8 DMA chunks of [6, 8192]
                xt_t = xp.tile([6, 8192], fmm, tag="xt")
                if k == 0:
                    # split so the first matmul's columns land early
                    nc.sync.dma_start(xt_t[:, 0:2048], xt[:, 0:2048])
                    nc.sync.dma_start(xt_t[:, 2048:8192], xt[:, 2048:8192])
                else:
                    nc.sync.dma_start(xt_t[:], xt[:, k * 8192 : (k + 1) * 8192])
                for m in range(4):
                    i = 4 * k + m
                    ps = pp.tile([128, 4, 16, 32], f32, tag="ps")
                    for b in range(4):
                        c0 = (4 * m + b) * 512
                        nc.tensor.matmul(
                            ps[:, b],
                            wpack_t[:],
                            xt_t[:, c0 : c0 + 512],
                        )
                    # all reduces on DVE: it is the only engine with a
                    # free-axis max on this target (Pool/GPSIMD rejects
                    # TensorTensor/TensorReduce/InstPool at the ISA level)
                    nc.vector.reduce_max(pooled[:, i], ps[:], axis=AX.X)
                    if i % 8 == 2 and i > 8:
                        # relu(+b1) an eighth of pooled once its chunks are
                        # done; deferred two chunks so the ACT-queue wait
                        # can't stall the next eviction.
                        s = i // 8 - 1
                        nc.scalar.activation(
                            pooledR[:, s * 512 : (s + 1) * 512],
                            pooled[:, s * 8 : (s + 1) * 8],
                            mybir.ActivationFunctionType.Relu,
                            bias=b1d_t[:],
                        )

            # last eighth of pooledR
            nc.scalar.activation(
                pooledR[:, 1536:2048],
                pooled[:, 24:32],
                mybir.ActivationFunctionType.Relu,
                bias=b1d_t[:],
            )

            # tail MLP, pipelined in 512-col sub-slices
            hps = pp.tile([128, 4, 16, 32], f32, tag="ps")
            hR = accp.tile([128, 2048], fmm, tag="hR")
            for j in range(4):
                nc.tensor.matmul(
                    hps[:, j],
                    w1gbd_t[:],
                    pooledR[:, j * 512 : (j + 1) * 512],
                )
                nc.scalar.activation(
                    hR[:, j * 512 : (j + 1) * 512],
                    hps[:, j],
                    mybir.ActivationFunctionType.Relu,
                    bias=b1gd_t[:],
                )

            opsA = pp.tile([128, 4, 16, 32], f32, tag="ps")
            opsB = pp.tile([128, 4, 16, 32], f32, tag="ps")
            o2A = accp.tile([128, 2048], f32, tag="o2A")
            o2B = accp.tile([128, 2048], f32, tag="o2B")
            add = mybir.AluOpType.add
            vmax = mybir.AluOpType.max
            for j in range(4):
                nc.tensor.matmul(
                    opsA[:, j],
                    w2gt_t[0:64, :],
                    hR[0:64, j * 512 : (j + 1) * 512],
                )
                nc.tensor.matmul(
                    opsB[:, j],
                    w2gt_t[64:128, :],
                    hR[64:128, j * 512 : (j + 1) * 512],
                )
                # relu(+b2g): o2A + first half of o2B on DVE, rest on ACT
                nc.vector.tensor_scalar(
                    o2A[:, j * 512 : (j + 1) * 512],
                    opsA[:, j], b2g_t[:], 0.0, op0=add, op1=vmax,
                )
                if j < 2:
                    nc.vector.tensor_scalar(
                        o2B[:, j * 512 : (j + 1) * 512],
                        opsB[:, j], b2g_t[:], 0.0, op0=add, op1=vmax,
                    )
                else:
                    nc.scalar.activation(
                        o2B[:, j * 512 : (j + 1) * 512],
                        opsB[:, j],
                        mybir.ActivationFunctionType.Relu,
                        bias=b2g_t[:],
                    )
                if j == 1:
                    nc.sync.dma_start(outA[:, 0:1024], o2A[:, 0:1024])
                if j == 2:
                    # after the j==2 ACT so the issue's wait on DVE's
                    # o2B slices can't stall ACT compute
                    nc.scalar.dma_start(outB[:, 0:1024], o2B[:, 0:1024])
            nc.sync.dma_start(outA[:, 1024:2048], o2A[:, 1024:2048])
            nc.scalar.dma_start(outB[:, 1024:2048], o2B[:, 1024:2048])

    nc.compile()
    return nc


def _get_program():
    global _PROGRAM
    if _PROGRAM is None:
        _PROGRAM = _build_program()
    return _PROGRAM


def _host_pack(relative_points, W1, b1, W1g, b1g, W2g, b2g):
    X = np.ascontiguousarray(relative_points, dtype=np.float32)
    W1 = np.asarray(W1, np.float32)
    b1 = np.asarray(b1, np.float32)
    W1g = np.asarray(W1g, np.float32)
    b1g = np.asarray(b1g, np.float32)
    W2g = np.asarray(W2g, np.float32)
    b2g = np.asarray(b2g, np.float32)

    wpack = np.zeros((6, 128), np.float32)
    wpack[0:3, 0:64] = W1.T
    wpack[3:6, 64:128] = W1.T
    b1d = np.concatenate([b1, b1]).reshape(128, 1)
    w1gbd = np.zeros((128, 128), np.float32)
    w1gbd[0:64, 0:64] = W1g.T
    w1gbd[64:128, 64:128] = W1g.T
    b1gd = np.concatenate([b1g, b1g]).reshape(128, 1)
    w2gt = np.ascontiguousarray(np.vstack([W2g.T, W2g.T]))  # [128, 128]
    b2gc = np.ascontiguousarray(b2g.reshape(128, 1))

    in_maps = []
    for d in range(NCORES):
        Xc = X[d * NPC : (d + 1) * NPC]
        xt6 = np.ascontiguousarray(
            Xc.reshape(G, 2, 512, 3).transpose(1, 3, 0, 2).reshape(6, G * 512)
        )
        in_maps.append(
            {
                "xt": xt6,
                "wpack": wpack,
                "b1d": b1d,
                "w1gbd": w1gbd,
                "b1gd": b1gd,
                "w2gt": w2gt,
                "b2g": b2gc,
            }
        )
    return in_maps


def _host_unpack(results):
    out = np.empty((S, FG1), np.float32)
    for d in range(NCORES):
        oA = results[d]["outA"].reshape(128, NCHUNK, 4, 16)
        oB = results[d]["outB"].reshape(128, NCHUNK, 4, 16)
        blk = out[d * SPC : (d + 1) * SPC].reshape(NCHUNK, 4, 2, 16, 128)
        blk[:, :, 0] = oA.transpose(1, 2, 3, 0)
        blk[:, :, 1] = oB.transpose(1, 2, 3, 0)
    return out


def _numpy_fallback(relative_points, cluster, num_clusters,
                    W1, b1, W1g, b1g, W2g, b2g):
    X = np.asarray(relative_points, np.float32)
    fc1 = np.maximum(X @ np.asarray(W1, np.float32).T + np.asarray(b1, np.float32), 0.0)
    Sn = int(num_clusters)
    cl = np.asarray(cluster).astype(np.int64)
    pooled = np.full((Sn, fc1.shape[1]), -np.inf, np.float32)
    # sorted segment ids -> reduceat over run starts
    starts = np.flatnonzero(np.r_[True, cl[1:] != cl[:-1]])
    seg_ids = cl[starts]
    pooled[seg_ids] = np.maximum.reduceat(fc1, starts, axis=0)
    h = np.maximum(pooled @ np.asarray(W1g, np.float32).T + np.asarray(b1g, np.float32), 0.0)
    return np.maximum(h @ np.asarray(W2g, np.float32).T + np.asarray(b2g, np.float32), 0.0).astype(np.float32)


def _run_hw(in_maps, trace=False):
    from concourse.bass_utils import run_bass_kernel_spmd

    nc = _get_program()
    return run_bass_kernel_spmd(
        nc, in_maps, list(range(NCORES)), trace=trace
    )


def kernel(relative_points, cluster, num_clusters,
           W1, b1, W1g, b1g, W2g, b2g):
    cl = np.asarray(cluster)
    expected_cl = np.arange(N, dtype=np.int64) // PTS_PER_CLUSTER
    if (
        relative_points.shape != (N, 3)
        or int(num_clusters) != S
        or not np.array_equal(cl, expected_cl)
    ):
        return _numpy_fallback(relative_points, cluster, num_clusters,
                               W1, b1, W1g, b1g, W2g, b2g)

    in_maps = _host_pack(relative_points, W1, b1, W1g, b1g, W2g, b2g)
    res = _run_hw(in_maps, trace=False)
    return _host_unpack(res.results)


def run_traced(inputs):
    """test.py helper: returns (output, exec_time_ns)."""
    in_maps = _host_pack(
        inputs["relative_points"], inputs["W1"], inputs["b1"],
        inputs["W1g"], inputs["b1g"], inputs["W2g"], inputs["b2g"],
    )
    res = _run_hw(in_maps, trace=True)
    return _host_unpack(res.results), res.exec_time_ns



# revision 3
# speedup vs baseline: 1.0617x; 1.0617x over previous
"""Trainium2 Bass kernel for OldNeighborhoodEncoder (segment_reduce).

Math (reference):
    fc1    = relu(X @ W1.T + b1)            # [N, 64], X = [N, 3]
    pooled = segment_max(fc1, cluster, S)   # [S, 64], cluster = arange(N)//32
    h      = relu(pooled @ W1g.T + b1g)     # [S, 64]
    out    = relu(h @ W2g.T + b2g)          # [S, 128]

Hardcoded sizes: N=1048576, S=32768 (32 pts/cluster), FEATURE=64, FG0=64,
FG1=128, 8 cores. Data-parallel over points: core d handles points
[d*131072, (d+1)*131072) == clusters [d*4096, (d+1)*4096); no collectives.

Device layout (per core):
  xt [6, 65536]: col c = 512*g + o (g in 0..127, o in 0..511); rows 0-2 =
    xyz of point 1024*g + o, rows 3-5 = xyz of point 1024*g + 512 + o.
  wpack [6,128] = blockdiag(W1.T, W1.T): one matmul column-block computes
    fc1 (pre-bias) for TWO 512-point chunks at once -> full 128-partition
    PE output. Bias+relu are deferred past the max (monotone).
  psum [128,4,16,32]: bank b holds g = 4i+b; view [.., q, t] with o=32q+t,
    so a single DVE reduce over t pools 4*16 = 64 cluster-halves.
  pooled [128, 32, 4, 16]: pooled[64a+f, i, b, q] = max_z of cluster
    128i + 32b + 16a + q, feature f.
  Tail: relu(+b1) -> blockdiag(W1g.T) matmul -> relu(+b1g) ->
    W2g.T matmul (K=64, separately for a=0 from partitions 0:64 and a=1
    from 64:128) -> relu(+b2g) -> outA/outB [128, 2048].

v1.5 perf structure: the main loop is DVE-reduce-bound (Pool/GPSIMD has no
legal max op on this target, so DVE does all 32 chunk reductions); weight
DMAs go on the Scalar queue (HWDGE; gpsimd SWDGE blocked the first matmul
~7us); relu(+b1) of pooled happens in slices during the main loop on ACT;
the tail MLP is pipelined in 512-col sub-slices with relu work split
between ACT and DVE, and output DMAs are split in halves on two queues.
"""

import sys
import numpy as np

if "/opt/trn_rl_repo" not in sys.path:
    sys.path.insert(0, "/opt/trn_rl_repo")

N = 1048576
S = 32768
PTS_PER_CLUSTER = 32
FEATURE = 64
FG0 = 64
FG1 = 128
NCORES = 8
NPC = N // NCORES          # 131072 points per core
SPC = S // NCORES          # 4096 clusters per core
G = NPC // 1024            # 128 column-groups of 512
NCHUNK = 32                # psum chunks per core (each = 4 groups)

_PROGRAM = None  # (nc, input_names) cache


def _build_program():
    from concourse import bacc, bass, tile

    mybir = bass.mybir
    f32 = mybir.dt.float32
    # bf16 matmul path: full-rate 1 cycle/row on PE (f32r measured ~3x
    # slower on hw despite the cost model's claim), halves the xt DMA.
    fmm = mybir.dt.bfloat16
    AX = mybir.AxisListType

    nc = bacc.Bacc("TRN2", target_bir_lowering=False, debug=False)

    xt = nc.dram_tensor("xt", [6, G * 512], fmm, kind="ExternalInput").ap()
    wpack = nc.dram_tensor("wpack", [6, 128], fmm, kind="ExternalInput").ap()
    b1d = nc.dram_tensor("b1d", [128, 1], f32, kind="ExternalInput").ap()
    w1gbd = nc.dram_tensor("w1gbd", [128, 128], fmm, kind="ExternalInput").ap()
    b1gd = nc.dram_tensor("b1gd", [128, 1], f32, kind="ExternalInput").ap()
    w2gt = nc.dram_tensor("w2gt", [128, 128], fmm, kind="ExternalInput").ap()
    b2g = nc.dram_tensor("b2g", [128, 1], f32, kind="ExternalInput").ap()
    outA = nc.dram_tensor("outA", [128, 2048], f32, kind="ExternalOutput").ap()
    outB = nc.dram_tensor("outB", [128, 2048], f32, kind="ExternalOutput").ap()

    with tile.TileContext(nc) as tc:
        with (
            tc.tile_pool(name="w", bufs=1) as wp,
            tc.tile_pool(name="x", bufs=3) as xp,
            tc.tile_pool(name="acc", bufs=1) as accp,
            tc.tile_pool(name="ps", bufs=2, space=bass.MemorySpace.PSUM) as pp,
        ):
            wpack_t = wp.tile([6, 128], fmm, tag="wpack")
            b1d_t = wp.tile([128, 1], f32, tag="b1d")
            w1gbd_t = wp.tile([128, 128], fmm, tag="w1gbd")
            b1gd_t = wp.tile([128, 1], f32, tag="b1gd")
            w2gt_t = wp.tile([128, 128], fmm, tag="w2gt")
            b2g_t = wp.tile([128, 1], f32, tag="b2g")
            # weight DMAs on the Scalar queue (HWDGE); wpack first — it
            # gates the first matmul.
            for t, d in (
                (wpack_t, wpack),
                (b1d_t, b1d),
                (w1gbd_t, w1gbd),
                (b1gd_t, b1gd),
                (w2gt_t, w2gt),
                (b2g_t, b2g),
            ):
                nc.scalar.dma_start(t[:], d[:])

            pooled = accp.tile([128, NCHUNK, 4, 16], f32, tag="pooled")
            pooledR = accp.tile([128, 2048], fmm, tag="pooledR")

            # main loop: fc1 matmuls + segment-max pooling
            for k in range(8):  # 8 DMA chunks of [6, 8192]
                xt_t = xp.tile([6, 8192], fmm, tag="xt")
                if k == 0:
                    # split so the first matmul's columns land early
                    nc.sync.dma_start(xt_t[:, 0:2048], xt[:, 0:2048])
                    nc.sync.dma_start(xt_t[:, 2048:8192], xt[:, 2048:8192])
                else:
                    nc.sync.dma_start(xt_t[:], xt[:, k * 8192 : (k + 1) * 8192])
                for m in range(4):
                    i = 4 * k + m
                    ps = pp.tile([128, 4, 16, 32], f32, tag="ps")
                    for b in range(4):
                        c0 = (4 * m + b) * 512
                        nc.tensor.matmul(
                            ps[:, b],
                            wpack_t[:],
                            xt_t[:, c0 : c0 + 512],
                        )
                    # all reduces on DVE: it is the only engine with a
                    # free-axis max on this target (Pool/GPSIMD rejects
                    # TensorTensor/TensorReduce/InstPool at the ISA level)
                    nc.vector.reduce_max(pooled[:, i], ps[:], axis=AX.X)
                    if i % 8 == 2 and i > 8:
                        # relu(+b1) an eighth of pooled once its chunks are
                        # done; deferred two chunks so the ACT-queue wait
                        # can't stall the next eviction.
                        s = i // 8 - 1
                        nc.scalar.activation(
                            pooledR[:, s * 512 : (s + 1) * 512],
                            pooled[:, s * 8 : (s + 1) * 8],
                            mybir.ActivationFunctionType.Relu,
                            bias=b1d_t[:],
                        )

            # last eighth of pooledR
            nc.scalar.activation(
                pooledR[:, 1536:2048],
                pooled[:, 24:32],
                mybir.ActivationFunctionType.Relu,
                bias=b1d_t[:],
            )

            # tail MLP, pipelined in 512-col sub-slices
            hps = pp.tile([128, 4, 16, 32], f32, tag="ps")
            hR = accp.tile([128, 2048], fmm, tag="hR")
            for j in range(4):
                nc.tensor.matmul(
                    hps[:, j],
                    w1gbd_t[:],
                    pooledR[:, j * 512 : (j + 1) * 512],
                )
                nc.scalar.activation(
                    hR[:, j * 512 : (j + 1) * 512],
                    hps[:, j],
                    mybir.ActivationFunctionType.Relu,
                    bias=b1gd_t[:],
                )

            opsA = pp.tile([128, 4, 16, 32], f32, tag="ps")
            opsB = pp.tile([128, 4, 16, 32], f32, tag="ps")
            o2A = accp.tile([128, 2048], f32, tag="o2A")
            o2B = accp.tile([128, 2048], f32, tag="o2B")
            add = mybir.AluOpType.add
            vmax = mybir.AluOpType.max
            for j in range(4):
                nc.tensor.matmul(
                    opsA[:, j],
                    w2gt_t[0:64, :],
                    hR[0:64, j * 512 : (j + 1) * 512],
                )
                nc.tensor.matmul(
                    opsB[:, j],
                    w2gt_t[64:128, :],
                    hR[64:128, j * 512 : (j + 1) * 512],
                )
                # relu(+b2g): o2A + first half of o2B on DVE, rest on ACT
                nc.vector.tensor_scalar(
                    o2A[:, j * 512 : (j + 1) * 512],
                    opsA[:, j], b2g_t[:], 0.0, op0=add, op1=vmax,
                )
                if j < 2:
                    nc.vector.tensor_scalar(
                        o2B[:, j * 512 : (j + 1) * 512],
                        opsB[:, j], b2g_t[:], 0.0, op0=add, op1=vmax,
                    )
                else:
                    nc.scalar.activation(
                        o2B[:, j * 512 : (j + 1) * 512],
                        opsB[:, j],
                        mybir.ActivationFunctionType.Relu,
                        bias=b2g_t[:],
                    )
                if j == 1:
                    nc.sync.dma_start(outA[:, 0:1024], o2A[:, 0:1024])
                if j == 2:
                    # after the j==2 ACT so the issue's wait on DVE's
                    # o2B slices can't stall ACT compute
                    nc.scalar.dma_start(outB[:, 0:1024], o2B[:, 0:1024])
            nc.sync.dma_start(outA[:, 1024:2048], o2A[:, 1024:2048])
            nc.scalar.dma_start(outB[:, 1024:2048], o2B[:, 1024:2048])

    nc.compile()
    return nc


def _get_program():
    global _PROGRAM
    if _PROGRAM is None:
        _PROGRAM = _build_program()
    return _PROGRAM


def _host_pack(relative_points, W1, b1, W1g, b1g, W2g, b2g):
    from ml_dtypes import bfloat16

    X = np.ascontiguousarray(relative_points, dtype=np.float32)
    W1 = np.asarray(W1, np.float32)
    b1 = np.asarray(b1, np.float32)
    W1g = np.asarray(W1g, np.float32)
    b1g = np.asarray(b1g, np.float32)
    W2g = np.asarray(W2g, np.float32)
    b2g = np.asarray(b2g, np.float32)

    wpack = np.zeros((6, 128), np.float32)
    wpack[0:3, 0:64] = W1.T
    wpack[3:6, 64:128] = W1.T
    wpack = wpack.astype(bfloat16)
    b1d = np.concatenate([b1, b1]).reshape(128, 1)
    w1gbd = np.zeros((128, 128), np.float32)
    w1gbd[0:64, 0:64] = W1g.T
    w1gbd[64:128, 64:128] = W1g.T
    w1gbd = w1gbd.astype(bfloat16)
    b1gd = np.concatenate([b1g, b1g]).reshape(128, 1)
    w2gt = np.ascontiguousarray(np.vstack([W2g.T, W2g.T])).astype(bfloat16)
    b2gc = np.ascontiguousarray(b2g.reshape(128, 1))

    in_maps = []
    for d in range(NCORES):
        Xc = X[d * NPC : (d + 1) * NPC]
        xt6 = np.ascontiguousarray(
            Xc.reshape(G, 2, 512, 3).transpose(1, 3, 0, 2).reshape(6, G * 512)
        ).astype(bfloat16)
        in_maps.append(
            {
                "xt": xt6,
                "wpack": wpack,
                "b1d": b1d,
                "w1gbd": w1gbd,
                "b1gd": b1gd,
                "w2gt": w2gt,
                "b2g": b2gc,
            }
        )
    return in_maps


def _host_unpack(results):
    out = np.empty((S, FG1), np.float32)
    for d in range(NCORES):
        oA = results[d]["outA"].reshape(128, NCHUNK, 4, 16)
        oB = results[d]["outB"].reshape(128, NCHUNK, 4, 16)
        blk = out[d * SPC : (d + 1) * SPC].reshape(NCHUNK, 4, 2, 16, 128)
        blk[:, :, 0] = oA.transpose(1, 2, 3, 0)
        blk[:, :, 1] = oB.transpose(1, 2, 3, 0)
    return out


def _numpy_fallback(relative_points, cluster, num_clusters,
                    W1, b1, W1g, b1g, W2g, b2g):
    X = np.asarray(relative_points, np.float32)
    fc1 = np.maximum(X @ np.asarray(W1, np.float32).T + np.asarray(b1, np.float32), 0.0)
    Sn = int(num_clusters)
    cl = np.asarray(cluster).astype(np.int64)
    pooled = np.full((Sn, fc1.shape[1]), -np.inf, np.float32)
    # sorted segment ids -> reduceat over run starts
    starts = np.flatnonzero(np.r_[True, cl[1:] != cl[:-1]])
    seg_ids = cl[starts]
    pooled[seg_ids] = np.maximum.reduceat(fc1, starts, axis=0)
    h = np.maximum(pooled @ np.asarray(W1g, np.float32).T + np.asarray(b1g, np.float32), 0.0)
    return np.maximum(h @ np.asarray(W2g, np.float32).T + np.asarray(b2g, np.float32), 0.0).astype(np.float32)


def _run_hw(in_maps, trace=False):
    from concourse.bass_utils import run_bass_kernel_spmd

    nc = _get_program()
    return run_bass_kernel_spmd(
        nc, in_maps, list(range(NCORES)), trace=trace
    )


def kernel(relative_points, cluster, num_clusters,
           W1, b1, W1g, b1g, W2g, b2g):
    cl = np.asarray(cluster)
    expected_cl = np.arange(N, dtype=np.int64) // PTS_PER_CLUSTER
    if (
        relative_points.shape != (N, 3)
        or int(num_clusters) != S
        or not np.array_equal(cl, expected_cl)
    ):
        return _numpy_fallback(relative_points, cluster, num_clusters,
                               W1, b1, W1g, b1g, W2g, b2g)

    in_maps = _host_pack(relative_points, W1, b1, W1g, b1g, W2g, b2g)
    res = _run_hw(in_maps, trace=False)
    return _host_unpack(res.results)


def run_traced(inputs):
    """test.py helper: returns (output, exec_time_ns)."""
    in_maps = _host_pack(
        inputs["relative_points"], inputs["W1"], inputs["b1"],
        inputs["W1g"], inputs["b1g"], inputs["W2g"], inputs["b2g"],
    )
    res = _run_hw(in_maps, trace=True)
    return _host_unpack(res.results), res.exec_time_ns



# revision 6
# speedup vs baseline: 1.1225x; 1.0574x over previous
"""Trainium2 Bass kernel for OldNeighborhoodEncoder (segment_reduce).

Math (reference):
    fc1    = relu(X @ W1.T + b1)            # [N, 64], X = [N, 3]
    pooled = segment_max(fc1, cluster, S)   # [S, 64], cluster = arange(N)//32
    h      = relu(pooled @ W1g.T + b1g)     # [S, 64]
    out    = relu(h @ W2g.T + b2g)          # [S, 128]

Hardcoded sizes: N=1048576, S=32768 (32 pts/cluster), FEATURE=64, FG0=64,
FG1=128, 8 cores. Data-parallel over points: core d handles points
[d*131072, (d+1)*131072) == clusters [d*4096, (d+1)*4096); no collectives.

Device layout (per core):
  xt [6, 65536]: col c = 512*g + o (g in 0..127, o in 0..511); rows 0-2 =
    xyz of point 1024*g + o, rows 3-5 = xyz of point 1024*g + 512 + o.
  wpack [6,128] = blockdiag(W1.T, W1.T): one matmul column-block computes
    fc1 (pre-bias) for TWO 512-point chunks at once -> full 128-partition
    PE output. Bias+relu are deferred past the max (monotone).
  psum [128,4,16,32]: bank b holds g = 4i+b; view [.., q, t] with o=32q+t,
    so a single DVE reduce over t pools 4*16 = 64 cluster-halves.
  pooled [128, 32, 4, 16]: pooled[64a+f, i, b, q] = max_z of cluster
    128i + 32b + 16a + q, feature f.
  Tail: relu(+b1) -> blockdiag(W1g.T) matmul -> relu(+b1g) ->
    W2g.T matmul (K=64, separately for a=0 from partitions 0:64 and a=1
    from 64:128) -> relu(+b2g) -> outA/outB [128, 2048].

v1.5 perf structure: the main loop is DVE-reduce-bound (Pool/GPSIMD has no
legal max op on this target, so DVE does all 32 chunk reductions); weight
DMAs go on the Scalar queue (HWDGE; gpsimd SWDGE blocked the first matmul
~7us); relu(+b1) of pooled happens in slices during the main loop on ACT;
the tail MLP is pipelined in 512-col sub-slices with relu work split
between ACT and DVE, and output DMAs are split in halves on two queues.
"""

import sys
import numpy as np

if "/opt/trn_rl_repo" not in sys.path:
    sys.path.insert(0, "/opt/trn_rl_repo")

N = 1048576
S = 32768
PTS_PER_CLUSTER = 32
FEATURE = 64
FG0 = 64
FG1 = 128
NCORES = 8
NPC = N // NCORES          # 131072 points per core
SPC = S // NCORES          # 4096 clusters per core
G = NPC // 1024            # 128 column-groups of 512
NCHUNK = 32                # psum chunks per core (each = 4 groups)

_PROGRAM = None  # (nc, input_names) cache


def _build_program():
    from concourse import bacc, bass, tile

    mybir = bass.mybir
    f32 = mybir.dt.float32
    # bf16 matmul path: full-rate 1 cycle/row on PE (f32r measured ~3x
    # slower on hw despite the cost model's claim), halves the xt DMA.
    fmm = mybir.dt.bfloat16
    AX = mybir.AxisListType

    nc = bacc.Bacc("TRN2", target_bir_lowering=False, debug=False)

    xt = nc.dram_tensor("xt", [6, G * 512], fmm, kind="ExternalInput").ap()
    wpack = nc.dram_tensor("wpack", [6, 128], fmm, kind="ExternalInput").ap()
    b1d = nc.dram_tensor("b1d", [128, 1], f32, kind="ExternalInput").ap()
    w1gbd = nc.dram_tensor("w1gbd", [128, 128], fmm, kind="ExternalInput").ap()
    b1gd = nc.dram_tensor("b1gd", [128, 1], f32, kind="ExternalInput").ap()
    w2gt = nc.dram_tensor("w2gt", [128, 128], fmm, kind="ExternalInput").ap()
    b2g = nc.dram_tensor("b2g", [128, 1], f32, kind="ExternalInput").ap()
    outA = nc.dram_tensor("outA", [128, 2048], f32, kind="ExternalOutput").ap()
    outB = nc.dram_tensor("outB", [128, 2048], f32, kind="ExternalOutput").ap()

    # chunks whose pooling runs as a direct f32 DVE reduce from PSUM; the
    # rest are relu(+b1)-copied PSUM->SBUF bf16 by ACT, then max-pooled on
    # DVE with a tensor_tensor tree (bf16 2x_1p: 2 results/cycle).
    D_CHUNKS = (2, 7, 11, 16, 20, 25, 29)
    d_index = {k: i for i, k in enumerate(D_CHUNKS)}

    Relu = mybir.ActivationFunctionType.Relu
    add = mybir.AluOpType.add
    vmax = mybir.AluOpType.max

    with tile.TileContext(nc) as tc:
        with (
            tc.tile_pool(name="w", bufs=1) as wp,
            tc.tile_pool(name="x", bufs=3) as xp,
            tc.tile_pool(name="pre", bufs=3) as prep,
            tc.tile_pool(name="scr", bufs=2) as scrp,
            tc.tile_pool(name="acc", bufs=1) as accp,
            tc.tile_pool(name="ps", bufs=2, space=bass.MemorySpace.PSUM) as pp,
        ):
            wpack_t = wp.tile([6, 128], fmm, tag="wpack")
            b1d_t = wp.tile([128, 1], f32, tag="b1d")
            w1gbd_t = wp.tile([128, 128], fmm, tag="w1gbd")
            b1gd_t = wp.tile([128, 1], f32, tag="b1gd")
            w2gt_t = wp.tile([128, 128], fmm, tag="w2gt")
            b2g_t = wp.tile([128, 1], f32, tag="b2g")
            # weight DMAs on the Scalar queue (HWDGE); wpack first — it
            # gates the first matmul.
            for t, d in (
                (wpack_t, wpack),
                (b1d_t, b1d),
                (w1gbd_t, w1gbd),
                (b1gd_t, b1gd),
                (w2gt_t, w2gt),
                (b2g_t, b2g),
            ):
                nc.scalar.dma_start(t[:], d[:])

            pooledF = accp.tile([128, len(D_CHUNKS), 64], f32, tag="pooledF")
            pooledR = accp.tile([128, 2048], fmm, tag="pooledR")

            def tt_tree(pt, sc, k):
                # 5-level pairwise-max tree: [128, 64, 32] bf16 -> pooledR
                # slice [128, 64]. Levels 1-4 run at 2 elem-results/cycle.
                l1 = sc[:, 0:1024].rearrange("p (g e) -> p g e", g=64)
                l2 = sc[:, 1024:1536].rearrange("p (g e) -> p g e", g=64)
                l3 = sc[:, 1536:1792].rearrange("p (g e) -> p g e", g=64)
                l4 = sc[:, 1792:1920].rearrange("p (g e) -> p g e", g=64)
                v = pt[:]
                nc.vector.tensor_tensor(l1, v[:, :, 0:16], v[:, :, 16:32], op=vmax)
                nc.vector.tensor_tensor(l2, l1[:, :, 0:8], l1[:, :, 8:16], op=vmax)
                nc.vector.tensor_tensor(l3, l2[:, :, 0:4], l2[:, :, 4:8], op=vmax)
                nc.vector.tensor_tensor(l4, l3[:, :, 0:2], l3[:, :, 2:4], op=vmax)
                nc.vector.tensor_tensor(
                    pooledR[:, k * 64 : (k + 1) * 64].rearrange(
                        "p (g e) -> p g e", g=64
                    ),
                    l4[:, :, 0:1],
                    l4[:, :, 1:2],
                    op=vmax,
                )

            # main loop: fc1 matmuls + split segment-max pooling.
            # pending = (kind, payload) deferred DVE/ACT post-ops, issued one
            # chunk late so a D-chunk's psum-freeing reduce is never queued
            # behind tree work on DVE.
            pending = []

            def flush_pending():
                kind, k, payload = pending.pop(0)
                if kind == "tree":
                    pt, sc = payload
                    tt_tree(pt, sc, k)
                else:  # relu+cast of a D-chunk's pooled slice
                    nc.scalar.activation(
                        pooledR[:, k * 64 : (k + 1) * 64],
                        pooledF[:, payload],
                        Relu,
                        bias=b1d_t[:],
                    )

            xt_t = None
            for k in range(NCHUNK):
                if k % 4 == 0:
                    xt_t = xp.tile([6, 8192], fmm, tag="xt")
                    c0 = k * 2048
                    if k == 0:
                        # split so the first matmul's columns land early
                        nc.sync.dma_start(xt_t[:, 0:2048], xt[:, 0:2048])
                        nc.sync.dma_start(xt_t[:, 2048:8192], xt[:, 2048:8192])
                    else:
                        nc.sync.dma_start(xt_t[:], xt[:, c0 : c0 + 8192])
                ps = pp.tile([128, 4, 16, 32], f32, tag="ps")
                for b in range(4):
                    c0 = (k % 4) * 2048 + b * 512
                    nc.tensor.matmul(
                        ps[:, b],
                        wpack_t[:],
                        xt_t[:, c0 : c0 + 512],
                    )
                if k in d_index:
                    di = d_index[k]
                    nc.vector.reduce_max(pooledF[:, di], ps[:], axis=AX.X)
                    pending.append(("relu", k, di))
                else:
                    pt = prep.tile([128, 64, 32], fmm, tag="pre")
                    sc = scrp.tile([128, 1920], fmm, tag="scr")
                    nc.scalar.activation(pt[:], ps[:], Relu, bias=b1d_t[:])
                    pending.append(("tree", k, (pt, sc)))
                if len(pending) > 1:
                    flush_pending()
            while pending:
                flush_pending()

            # tail MLP: h = relu(pooledR @ W1g.T + b1g), out = relu(h @ ...)
            hps = pp.tile([128, 4, 16, 32], f32, tag="ps")
            hR = accp.tile([128, 2048], fmm, tag="hR")
            for j in range(4):
                nc.tensor.matmul(
                    hps[:, j],
                    w1gbd_t[:],
                    pooledR[:, j * 512 : (j + 1) * 512],
                )
                nc.scalar.activation(
                    hR[:, j * 512 : (j + 1) * 512],
                    hps[:, j],
                    Relu,
                    bias=b1gd_t[:],
                )

            o2A = accp.tile([128, 2048], f32, tag="o2A")
            o2B = accp.tile([128, 2048], f32, tag="o2B")
            for jj in range(2):
                ops = pp.tile([128, 4, 16, 32], f32, tag="ps")
                for m in range(2):
                    j = 2 * jj + m
                    nc.tensor.matmul(
                        ops[:, 2 * m],
                        w2gt_t[0:64, :],
                        hR[0:64, j * 512 : (j + 1) * 512],
                    )
                    nc.tensor.matmul(
                        ops[:, 2 * m + 1],
                        w2gt_t[64:128, :],
                        hR[64:128, j * 512 : (j + 1) * 512],
                    )
                    nc.vector.tensor_scalar(
                        o2A[:, j * 512 : (j + 1) * 512],
                        ops[:, 2 * m], b2g_t[:], 0.0, op0=add, op1=vmax,
                    )
                    nc.scalar.activation(
                        o2B[:, j * 512 : (j + 1) * 512],
                        ops[:, 2 * m + 1],
                        Relu,
                        bias=b2g_t[:],
                    )
                nc.sync.dma_start(
                    outA[:, jj * 1024 : (jj + 1) * 1024],
                    o2A[:, jj * 1024 : (jj + 1) * 1024],
                )
                nc.scalar.dma_start(
                    outB[:, jj * 1024 : (jj + 1) * 1024],
                    o2B[:, jj * 1024 : (jj + 1) * 1024],
                )

    nc.compile()
    return nc


def _get_program():
    global _PROGRAM
    if _PROGRAM is None:
        _PROGRAM = _build_program()
    return _PROGRAM


def _host_pack(relative_points, W1, b1, W1g, b1g, W2g, b2g):
    from ml_dtypes import bfloat16

    X = np.ascontiguousarray(relative_points, dtype=np.float32)
    W1 = np.asarray(W1, np.float32)
    b1 = np.asarray(b1, np.float32)
    W1g = np.asarray(W1g, np.float32)
    b1g = np.asarray(b1g, np.float32)
    W2g = np.asarray(W2g, np.float32)
    b2g = np.asarray(b2g, np.float32)

    wpack = np.zeros((6, 128), np.float32)
    wpack[0:3, 0:64] = W1.T
    wpack[3:6, 64:128] = W1.T
    wpack = wpack.astype(bfloat16)
    b1d = np.concatenate([b1, b1]).reshape(128, 1)
    w1gbd = np.zeros((128, 128), np.float32)
    w1gbd[0:64, 0:64] = W1g.T
    w1gbd[64:128, 64:128] = W1g.T
    w1gbd = w1gbd.astype(bfloat16)
    b1gd = np.concatenate([b1g, b1g]).reshape(128, 1)
    w2gt = np.ascontiguousarray(np.vstack([W2g.T, W2g.T])).astype(bfloat16)
    b2gc = np.ascontiguousarray(b2g.reshape(128, 1))

    in_maps = []
    for d in range(NCORES):
        Xc = X[d * NPC : (d + 1) * NPC]
        xt6 = np.ascontiguousarray(
            Xc.reshape(G, 2, 512, 3).transpose(1, 3, 0, 2).reshape(6, G * 512)
        ).astype(bfloat16)
        in_maps.append(
            {
                "xt": xt6,
                "wpack": wpack,
                "b1d": b1d,
                "w1gbd": w1gbd,
                "b1gd": b1gd,
                "w2gt": w2gt,
                "b2g": b2gc,
            }
        )
    return in_maps


def _host_unpack(results):
    out = np.empty((S, FG1), np.float32)
    for d in range(NCORES):
        oA = results[d]["outA"].reshape(128, NCHUNK, 4, 16)
        oB = results[d]["outB"].reshape(128, NCHUNK, 4, 16)
        blk = out[d * SPC : (d + 1) * SPC].reshape(NCHUNK, 4, 2, 16, 128)
        blk[:, :, 0] = oA.transpose(1, 2, 3, 0)
        blk[:, :, 1] = oB.transpose(1, 2, 3, 0)
    return out


def _numpy_fallback(relative_points, cluster, num_clusters,
                    W1, b1, W1g, b1g, W2g, b2g):
    X = np.asarray(relative_points, np.float32)
    fc1 = np.maximum(X @ np.asarray(W1, np.float32).T + np.asarray(b1, np.float32), 0.0)
    Sn = int(num_clusters)
    cl = np.asarray(cluster).astype(np.int64)
    pooled = np.full((Sn, fc1.shape[1]), -np.inf, np.float32)
    # sorted segment ids -> reduceat over run starts
    starts = np.flatnonzero(np.r_[True, cl[1:] != cl[:-1]])
    seg_ids = cl[starts]
    pooled[seg_ids] = np.maximum.reduceat(fc1, starts, axis=0)
    h = np.maximum(pooled @ np.asarray(W1g, np.float32).T + np.asarray(b1g, np.float32), 0.0)
    return np.maximum(h @ np.asarray(W2g, np.float32).T + np.asarray(b2g, np.float32), 0.0).astype(np.float32)


def _run_hw(in_maps, trace=False):
    from concourse.bass_utils import run_bass_kernel_spmd

    nc = _get_program()
    return run_bass_kernel_spmd(
        nc, in_maps, list(range(NCORES)), trace=trace
    )


def kernel(relative_points, cluster, num_clusters,
           W1, b1, W1g, b1g, W2g, b2g):
    cl = np.asarray(cluster)
    expected_cl = np.arange(N, dtype=np.int64) // PTS_PER_CLUSTER
    if (
        relative_points.shape != (N, 3)
        or int(num_clusters) != S
        or not np.array_equal(cl, expected_cl)
    ):
        return _numpy_fallback(relative_points, cluster, num_clusters,
                               W1, b1, W1g, b1g, W2g, b2g)

    in_maps = _host_pack(relative_points, W1, b1, W1g, b1g, W2g, b2g)
    res = _run_hw(in_maps, trace=False)
    return _host_unpack(res.results)


def run_traced(inputs):
    """test.py helper: returns (output, exec_time_ns)."""
    in_maps = _host_pack(
        inputs["relative_points"], inputs["W1"], inputs["b1"],
        inputs["W1g"], inputs["b1g"], inputs["W2g"], inputs["b2g"],
    )
    res = _run_hw(in_maps, trace=True)
    return _host_unpack(res.results), res.exec_time_ns

